# revision 15
# baseline (speedup 1.0000x reference)
"""MoE expert-group kernel for 8 Trainium2 NeuronCores.

Strategy (expert-parallel, per the sharding hint):
  - Host computes the (tiny) router: logits = x @ Wg.T, top-2, softmax.
  - Tokens are gathered per expert on host ("dispatch"). Each core owns
    EPC=2 expert *slots*; experts are assigned to slots balanced by load:
    slot 0 holds one of the 8 busiest experts (capacity capA = max of
    their loads), slot 1 one of the 8 lightest (capB). This beats the
    fixed (2c, 2c+1) pairing: per-core padded work drops from
    2*ceil(max_e load) to capA+capB.
  - Each core runs a dense 2-layer MLP (relu(x@W1+b1)@W2+b2) over its
    gathered tokens in transposed layout: weights are the stationary
    matmul operand, activations stream as the moving operand, biases are
    per-partition activation biases.
  - Host applies the per-(token, expert) softmax weight and scatter-adds
    ("combine") back to the full [8192, 1024] output.

Matmul dtype: bf16 operands (weights AND activations) with fp32 PSUM
accumulation. Measured on this part, fp32r sustains only ~0.86 ns/row
(~1.2GHz effective) while bf16 hits the full 2.4GHz rate; bf16 also
halves DMA. rel-err stays ~1e-3, well inside the 2e-2 gate.
"""

import os
import sys
import time

import numpy as np

sys.path.insert(0, "/opt/trn_rl_repo")

N_TOKENS = 8192
D_MODEL = 1024
D_HIDDEN = 2048
N_EXPERTS = 16
TOP_K = 2
N_CORES = 8
EPC = N_EXPERTS // N_CORES  # expert slots per core
KC1 = D_MODEL // 128   # k-chunks layer 1
MC1 = D_HIDDEN // 128  # m-chunks layer 1
KC2 = D_HIDDEN // 128  # k-chunks layer 2
MC2 = D_MODEL // 128   # m-chunks layer 2

# matmul dtype mode: "fp32" (bit-exact-ish, 1/4 rate), "fp32r" (fp32
# operands, reduced-precision multiplies), "bf16" (bf16 operands)
MM_MODE = os.environ.get("KERNEL_MM_MODE", "bf16")
USE_512_TILES = os.environ.get("KERNEL_512_TILES", "1") == "1"
# Layer-2 layout B: moving dim = output features (all 512-wide matmuls,
# ~25% fewer PE instructions; the PE is issue-limited at ~287ns/matmul).
L2B = os.environ.get("KERNEL_L2B", "1") == "1"


def _split_tiles(cap):
    """Split cap into matmul moving-dim tiles: 512 where cap aligns (fewer
    instructions -> less per-instruction overhead), else <=384 and >=256,
    all multiples of 8 (the regime validated by the PE probes)."""
    if USE_512_TILES and cap % 512 == 0:
        return [512] * (cap // 512)
    n = max(1, -(-cap // 384))
    assert cap % 8 == 0, cap
    base = (cap // n) // 8 * 8
    tiles = [base] * n
    rem = cap - base * n
    i = 0
    while rem > 0:
        add = min(8, rem)
        tiles[i % n] += add
        rem -= add
        i += 1
    assert sum(tiles) == cap and all(256 <= t <= 384 for t in tiles), (cap, tiles)
    # ascending: the last (largest) tile maximizes the compute window that
    # hides the next slot's weight prefetch
    return sorted(tiles)


def build_program(caps, mode=MM_MODE, loop_reps=1):
    """Build the per-core program. caps: per-slot token capacities
    (len EPC). loop_reps>1 wraps the body in a hardware For_i loop
    (identical work each iteration) for wall-clock timing."""
    import contextlib

    import concourse.mybir as mybir
    import concourse.tile as tile
    from concourse import bacc

    f32 = mybir.dt.float32
    if mode == "fp32":
        act_dt = w_dt = f32
    elif mode == "fp32r":
        act_dt = w_dt = mybir.dt.float32r
    elif mode == "bf16":
        act_dt = w_dt = mybir.dt.bfloat16
    else:
        raise ValueError(mode)

    nc = bacc.Bacc("TRN2", target_bir_lowering=False, debug=False)
    xts, w1s, b1s, w2s, b2s, yts = [], [], [], [], [], []
    for s, cap in enumerate(caps):
        xts.append(
            nc.dram_tensor(f"xt{s}", [D_MODEL, cap], act_dt, kind="ExternalInput").ap()
        )
        w1s.append(
            nc.dram_tensor(
                f"w1{s}", [D_MODEL, D_HIDDEN], w_dt, kind="ExternalInput"
            ).ap()
        )
        b1s.append(
            nc.dram_tensor(f"b1{s}", [D_HIDDEN], f32, kind="ExternalInput").ap()
        )
        w2s.append(
            nc.dram_tensor(
                f"w2{s}", [D_HIDDEN, D_MODEL], w_dt, kind="ExternalInput"
            ).ap()
        )
        if L2B:
            # b2 replicated across partitions (bias varies along the free
            # dim in layout B, so it is added on the DVE, not the act bias)
            b2s.append(
                nc.dram_tensor(
                    f"b2r{s}", [128, D_MODEL], f32, kind="ExternalInput"
                ).ap()
            )
            tc_n = -(-cap // 128)
            yts.append(
                nc.dram_tensor(
                    f"yt{s}", [tc_n * 128, D_MODEL], f32, kind="ExternalOutput"
                ).ap()
            )
        else:
            b2s.append(
                nc.dram_tensor(f"b2{s}", [D_MODEL], f32, kind="ExternalInput").ap()
            )
            yts.append(
                nc.dram_tensor(
                    f"yt{s}", [D_MODEL, cap], f32, kind="ExternalOutput"
                ).ap()
            )

    Relu = mybir.ActivationFunctionType.Relu
    Ident = mybir.ActivationFunctionType.Identity

    HALF = MC2 // 2  # m2 chunks per W2 half-pool
    with tile.TileContext(nc) as tc:
        with (
            tc.tile_pool(name="w1pa", bufs=1) as w1pa,
            tc.tile_pool(name="w1pb", bufs=1) as w1pb,
            tc.tile_pool(name="w2pa", bufs=1) as w2pa,
            tc.tile_pool(name="w2pb", bufs=1) as w2pb,
            tc.tile_pool(name="bp", bufs=2) as bp,
            tc.tile_pool(name="bpr", bufs=2) as bpr,
            tc.tile_pool(name="xp", bufs=2) as xp,
            tc.tile_pool(name="hp", bufs=2) as hp,
            tc.tile_pool(name="yp", bufs=4) as yp,
            tc.tile_pool(name="ps1", bufs=2, space="PSUM") as ps1,
            tc.tile_pool(name="ps2", bufs=4 if L2B else 2, space="PSUM") as ps2,
        ):
            loop_cm = (
                tc.For_i(0, loop_reps, 1)
                if loop_reps > 1
                else contextlib.nullcontext()
            )
            with loop_cm:
                for e, cap in enumerate(caps):
                    tiles = _split_tiles(cap)
                    xt_src = xts[e].rearrange("(c p) n -> p c n", p=128)
                    if L2B:
                        yt_dst = yts[e].rearrange("(t p) d -> p t d", p=128)
                    else:
                        yt_dst = yts[e].rearrange("(c p) n -> p c n", p=128)

                    # The DMA fabric drains transfers roughly in enqueue
                    # order, so emissions follow need order. W1: two
                    # half-pools (m 0..7 / 8..15) x quarter-DMAs each on
                    # sync; quarters keep startup short, half-pools release
                    # early so the next slot's W1 hides under compute.
                    # Biases slot between quarters (tiny, needed early).
                    w1_src = w1s[e].rearrange("(c p) m -> p c m", p=128)
                    HW1 = D_HIDDEN // 2
                    w1ta = w1pa.tile([128, KC1, HW1], w_dt, tag="w1ta")
                    w1tb = w1pb.tile([128, KC1, HW1], w_dt, tag="w1tb")
                    NQ1 = 4  # DMAs per W1 half-pool
                    EW = HW1 // NQ1
                    nc.sync.dma_start(w1ta[:, :, :EW], w1_src[:, :, :EW])
                    b1t = bp.tile([128, MC1], f32, tag="b1t")
                    nc.sync.dma_start(b1t[:], b1s[e].rearrange("(m p) -> p m", p=128))
                    if L2B:
                        b2t = bpr.tile([128, D_MODEL], f32, tag="b2rt")
                        nc.sync.dma_start(b2t[:], b2s[e])
                    else:
                        b2t = bp.tile([128, MC2], f32, tag="b2t")
                        nc.sync.dma_start(
                            b2t[:], b2s[e].rearrange("(m p) -> p m", p=128)
                        )
                    for q in range(1, NQ1):
                        nc.sync.dma_start(
                            w1ta[:, :, q * EW : (q + 1) * EW],
                            w1_src[:, :, q * EW : (q + 1) * EW],
                        )
                    for q in range(NQ1):
                        nc.sync.dma_start(
                            w1tb[:, :, q * EW : (q + 1) * EW],
                            w1_src[:, :, HW1 + q * EW : HW1 + (q + 1) * EW],
                        )

                    # gpsimd queue in need order: xt[0], W2 quarters (into
                    # two half-pools, so the next slot's halves load under
                    # this slot's L2), then xt[1..].
                    w2_src = w2s[e].rearrange("(c p) m -> p c m", p=128)
                    off = [sum(tiles[:j]) for j in range(len(tiles))]
                    xtiles = []
                    for j, nt in enumerate(tiles):
                        xtile_j = xp.tile(
                            [128, KC1, nt], act_dt, tag="xtile", name=f"xtile_{e}_{j}"
                        )
                        xtiles.append(xtile_j)
                    # xt[0] split by k-chunk pairs: the first L1 matmul only
                    # needs k-chunk 0, so it starts early.
                    for cc in range(0, KC1, 2):
                        nc.gpsimd.dma_start(
                            xtiles[0][:, cc : cc + 2, :],
                            xt_src[:, cc : cc + 2, off[0] : off[0] + tiles[0]],
                        )
                    if len(tiles) > 1:
                        nc.gpsimd.dma_start(
                            xtiles[1][:], xt_src[:, :, off[1] : off[1] + tiles[1]]
                        )
                    HW2 = HALF * 128
                    w2ta = w2pa.tile([128, KC2, HW2], w_dt, tag="w2ta")
                    w2tb = w2pb.tile([128, KC2, HW2], w_dt, tag="w2tb")
                    NQ2 = 4  # DMAs per W2 half-pool
                    QW2 = HW2 // NQ2
                    for q in range(NQ2):
                        nc.gpsimd.dma_start(
                            w2ta[:, :, q * QW2 : (q + 1) * QW2],
                            w2_src[:, :, q * QW2 : (q + 1) * QW2],
                        )
                    for q in range(NQ2):
                        nc.gpsimd.dma_start(
                            w2tb[:, :, q * QW2 : (q + 1) * QW2],
                            w2_src[:, :, HW2 + q * QW2 : HW2 + (q + 1) * QW2],
                        )
                    for j in range(2, len(tiles)):
                        nc.gpsimd.dma_start(
                            xtiles[j][:], xt_src[:, :, off[j] : off[j] + tiles[j]]
                        )

                    # j-level software pipeline: L1(0), L1(1), L2(0...),
                    # L1(2), L2(...) — PE is in-order, so emitting the
                    # next tile's L1 before this tile's L2 lets layer-1 run
                    # while W2 is still streaming in.
                    T = len(tiles)
                    if L2B:
                        # unified ht per slot: L2B's 128-token lhsT slices
                        # cross j-tile boundaries
                        ht_u = hp.tile(
                            [128, KC2, cap], act_dt, tag="ht", name=f"ht_{e}"
                        )
                        hts = [None] * T
                    else:
                        hts = [None] * T

                    def layer1(j):
                        nt = tiles[j]
                        if L2B:
                            ht = None
                        else:
                            ht = hp.tile(
                                [128, KC2, nt], act_dt, tag="ht", name=f"ht_{e}_{j}"
                            )
                            hts[j] = ht
                        for m in range(MC1):
                            w1h = w1ta if m < MC1 // 2 else w1tb
                            mh = m % (MC1 // 2)
                            hps = ps1.tile([128, nt], f32, tag="hps")
                            for c in range(KC1):
                                nc.tensor.matmul(
                                    hps[:],
                                    lhsT=w1h[:, c, mh * 128 : (mh + 1) * 128],
                                    rhs=xtiles[j][:, c, :],
                                    start=(c == 0),
                                    stop=(c == KC1 - 1),
                                )
                            if L2B:
                                nc.scalar.activation(
                                    ht_u[:, m, off[j] : off[j] + nt],
                                    hps[:],
                                    Relu,
                                    bias=b1t[:, m : m + 1],
                                )
                            else:
                                nc.scalar.activation(
                                    ht[:, m, :], hps[:], Relu, bias=b1t[:, m : m + 1]
                                )

                    def layer2(j):
                        nt = tiles[j]
                        ht = hts[j]
                        for m in range(MC2):
                            w2h = w2ta if m < HALF else w2tb
                            mh = m % HALF
                            yps = ps2.tile([128, nt], f32, tag="yps")
                            for c in range(KC2):
                                nc.tensor.matmul(
                                    yps[:],
                                    lhsT=w2h[:, c, mh * 128 : (mh + 1) * 128],
                                    rhs=ht[:, c, :],
                                    start=(c == 0),
                                    stop=(c == KC2 - 1),
                                )
                            ysb = yp.tile([128, nt], f32, tag="ysb")
                            nc.scalar.activation(
                                ysb[:], yps[:], Ident, bias=b2t[:, m : m + 1]
                            )
                            nc.scalar.dma_start(
                                yt_dst[:, m, off[j] : off[j] + nt], ysb[:]
                            )

                    def layer2b(tc_list):
                        # layout B: out[tokens, d] — lhsT = ht 128-token
                        # slice, rhs = W2 rows, moving dim = d (512-wide)
                        Add = mybir.AluOpType.add
                        Mult = mybir.AluOpType.mult
                        for tcn in tc_list:
                            pw = min(128, cap - tcn * 128)
                            for dseg in range(2):
                                w2h = w2ta if dseg == 0 else w2tb
                                yps = ps2.tile([128, 512], f32, tag="yps")
                                for hk in range(KC2):
                                    nc.tensor.matmul(
                                        yps[:pw, :],
                                        lhsT=ht_u[:, hk, tcn * 128 : tcn * 128 + pw],
                                        rhs=w2h[:, hk, :],
                                        start=(hk == 0),
                                        stop=(hk == KC2 - 1),
                                    )
                                ysb = yp.tile([128, 512], f32, tag="ysb")
                                nc.vector.scalar_tensor_tensor(
                                    ysb[:pw, :],
                                    yps[:pw, :],
                                    1.0,
                                    b2t[:pw, dseg * 512 : (dseg + 1) * 512],
                                    Mult,
                                    Add,
                                )
                                nc.scalar.dma_start(
                                    yt_dst[:pw, tcn, dseg * 512 : (dseg + 1) * 512],
                                    ysb[:pw, :],
                                )

                    if L2B:
                        TC = -(-cap // 128)
                        # tc chunk usable once L1 has covered its tokens
                        cum = 0
                        ready_after = []  # per j: last tc (exclusive) ready
                        for j, nt in enumerate(tiles):
                            cum += nt
                            ready_after.append(
                                TC if cum == cap else cum // 128
                            )
                        emitted = 0
                        for j in range(T):
                            layer1(j)
                            if j >= 1:
                                layer2b(range(emitted, ready_after[j - 1]))
                                emitted = ready_after[j - 1]
                        layer2b(range(emitted, TC))
                    else:
                        for k in range(T + 1):
                            if k < T:
                                layer1(k)
                            if k >= 1:
                                layer2(k - 1)
    nc.compile()
    return nc


def route(x, Wg):
    """Host router identical (up to fp rounding far below the top-2/3
    logit gap) to the reference: top-2 by logit, softmax over the pair."""
    logits = x.astype(np.float32, copy=False) @ Wg.astype(np.float32, copy=False).T
    n = logits.shape[0]
    rows = np.arange(n)
    i1 = np.argmax(logits, axis=1)
    v1 = logits[rows, i1]
    masked = logits.copy()
    masked[rows, i1] = -np.inf
    i2 = np.argmax(masked, axis=1)
    v2 = masked[rows, i2]
    d = np.exp((v2 - v1).astype(np.float64))
    wt1 = (1.0 / (1.0 + d)).astype(np.float32)
    wt2 = (d / (1.0 + d)).astype(np.float32)
    return i1, i2, wt1, wt2


def plan(x, Wg):
    """Route, then assign experts to (core, slot) balanced by load.

    Returns (idxs, wts, slot_expert, caps):
      idxs[e], wts[e]: token ids / combine weights per expert
      slot_expert[core][s]: expert id in slot s of core
      caps[s]: token capacity of slot s (same across cores)
    """
    i1, i2, wt1, wt2 = route(x, Wg)
    idxs, wts = [], []
    for e in range(N_EXPERTS):
        sel1 = i1 == e
        sel2 = i2 == e
        idx = np.concatenate([np.nonzero(sel1)[0], np.nonzero(sel2)[0]])
        w = np.concatenate([wt1[sel1], wt2[sel2]])
        idxs.append(idx)
        wts.append(w)

    loads = np.array([len(i) for i in idxs])
    order = np.argsort(-loads)  # experts by load, desc
    slot_expert = [[0] * EPC for _ in range(N_CORES)]
    caps = []
    for s in range(EPC):
        grp = order[s * N_CORES : (s + 1) * N_CORES]
        for core in range(N_CORES):
            slot_expert[core][s] = int(grp[core])
        cap = int(loads[grp].max())
        cap = max(256, -(-cap // 16) * 16)  # >=256 (tile floor), mult of 16
        caps.append(cap)
    return idxs, wts, slot_expert, caps


def make_in_maps(x, W1, b1, W2, b2, idxs, slot_expert, caps, mode=None):
    import ml_dtypes

    if mode is None:
        mode = MM_MODE
    np_dt = ml_dtypes.bfloat16 if mode == "bf16" else np.float32
    in_maps = []
    for core in range(N_CORES):
        m = {}
        for s, cap in enumerate(caps):
            e = slot_expert[core][s]
            xt = np.zeros((D_MODEL, cap), dtype=np_dt)
            xt[:, : len(idxs[e])] = x[idxs[e]].T.astype(np_dt)
            m[f"xt{s}"] = xt
            m[f"w1{s}"] = np.ascontiguousarray(W1[e]).astype(np_dt)
            m[f"b1{s}"] = np.ascontiguousarray(b1[e])
            m[f"w2{s}"] = np.ascontiguousarray(W2[e]).astype(np_dt)
            if L2B:
                m[f"b2r{s}"] = np.ascontiguousarray(
                    np.broadcast_to(b2[e], (128, D_MODEL)).astype(np.float32)
                )
            else:
                m[f"b2{s}"] = np.ascontiguousarray(b2[e])
        in_maps.append(m)
    return in_maps


def kernel(x, Wg, W1, b1, W2, b2):
    from concourse.bass_utils import run_bass_kernel_spmd

    x = np.ascontiguousarray(np.asarray(x, dtype=np.float32))
    Wg = np.asarray(Wg, dtype=np.float32)
    W1 = np.asarray(W1, dtype=np.float32)
    b1 = np.asarray(b1, dtype=np.float32)
    W2 = np.asarray(W2, dtype=np.float32)
    b2 = np.asarray(b2, dtype=np.float32)
    n_tokens = x.shape[0]

    idxs, wts, slot_expert, caps = plan(x, Wg)
    in_maps = make_in_maps(x, W1, b1, W2, b2, idxs, slot_expert, caps)

    nc = build_program(caps)
    res = run_bass_kernel_spmd(nc, in_maps, core_ids=list(range(N_CORES)))

    out = np.zeros((n_tokens, D_MODEL), dtype=np.float32)
    for core in range(N_CORES):
        for s in range(EPC):
            e = slot_expert[core][s]
            n_e = len(idxs[e])
            if n_e == 0:
                continue
            if L2B:
                y = res.results[core][f"yt{s}"][:n_e]  # [n_e, D]
            else:
                y = res.results[core][f"yt{s}"][:, :n_e].T  # [n_e, D]
            out[idxs[e]] += wts[e][:, None] * y
    return out


if __name__ == "__main__":
    rng = np.random.default_rng(0)
    x = rng.standard_normal((N_TOKENS, D_MODEL), dtype=np.float32)
    s_in = 1.0 / np.sqrt(D_MODEL)
    s_hid = 1.0 / np.sqrt(D_HIDDEN)
    Wg = rng.uniform(-s_in, s_in, (N_EXPERTS, D_MODEL)).astype(np.float32)
    W1 = rng.uniform(-s_in, s_in, (N_EXPERTS, D_MODEL, D_HIDDEN)).astype(np.float32)
    b1 = rng.uniform(-s_in, s_in, (N_EXPERTS, D_HIDDEN)).astype(np.float32)
    W2 = rng.uniform(-s_hid, s_hid, (N_EXPERTS, D_HIDDEN, D_MODEL)).astype(np.float32)
    b2 = rng.uniform(-s_hid, s_hid, (N_EXPERTS, D_MODEL)).astype(np.float32)
    t0 = time.time()
    out = kernel(x=x, Wg=Wg, W1=W1, b1=b1, W2=W2, b2=b2)
    print("kernel() wall:", time.time() - t0, "out", out.shape, out.dtype)


# revision 16
# speedup vs baseline: 1.1386x; 1.1386x over previous
"""MoE expert-group kernel for 8 Trainium2 NeuronCores.

Strategy (expert-parallel, per the sharding hint):
  - Host computes the (tiny) router: logits = x @ Wg.T, top-2, softmax.
  - Tokens are gathered per expert on host ("dispatch"). Each core owns
    EPC=2 expert *slots*; experts are assigned to slots balanced by load:
    slot 0 holds one of the 8 busiest experts (capacity capA = max of
    their loads), slot 1 one of the 8 lightest (capB). This beats the
    fixed (2c, 2c+1) pairing: per-core padded work drops from
    2*ceil(max_e load) to capA+capB.
  - Each core runs a dense 2-layer MLP (relu(x@W1+b1)@W2+b2) over its
    gathered tokens in transposed layout: weights are the stationary
    matmul operand, activations stream as the moving operand, biases are
    per-partition activation biases.
  - Host applies the per-(token, expert) softmax weight and scatter-adds
    ("combine") back to the full [8192, 1024] output.

Matmul dtype: bf16 operands (weights AND activations) with fp32 PSUM
accumulation. Measured on this part, fp32r sustains only ~0.86 ns/row
(~1.2GHz effective) while bf16 hits the full 2.4GHz rate; bf16 also
halves DMA. rel-err stays ~1e-3, well inside the 2e-2 gate.
"""

import os
import sys
import time

import numpy as np

sys.path.insert(0, "/opt/trn_rl_repo")

N_TOKENS = 8192
D_MODEL = 1024
D_HIDDEN = 2048
N_EXPERTS = 16
TOP_K = 2
N_CORES = 8
EPC = N_EXPERTS // N_CORES  # expert slots per core
KC1 = D_MODEL // 128   # k-chunks layer 1
MC1 = D_HIDDEN // 128  # m-chunks layer 1
KC2 = D_HIDDEN // 128  # k-chunks layer 2
MC2 = D_MODEL // 128   # m-chunks layer 2

# matmul dtype mode: "fp32" (bit-exact-ish, 1/4 rate), "fp32r" (fp32
# operands, reduced-precision multiplies), "bf16" (bf16 operands)
MM_MODE = os.environ.get("KERNEL_MM_MODE", "bf16")
USE_512_TILES = os.environ.get("KERNEL_512_TILES", "1") == "1"
# Layer-2 layout B: moving dim = output features (fewer PE instructions,
# but measured slower in situ than layout A — unified-ht dependencies
# outweigh the instruction-count savings). Keep off.
L2B = os.environ.get("KERNEL_L2B", "0") == "1"


def _split_tiles(cap):
    """Split cap into matmul moving-dim tiles: 512 where cap aligns (fewer
    instructions -> less per-instruction overhead), else <=384 and >=256,
    all multiples of 8 (the regime validated by the PE probes)."""
    if USE_512_TILES and cap % 512 == 0:
        return [512] * (cap // 512)
    n = max(1, -(-cap // 384))
    assert cap % 8 == 0, cap
    base = (cap // n) // 8 * 8
    tiles = [base] * n
    rem = cap - base * n
    i = 0
    while rem > 0:
        add = min(8, rem)
        tiles[i % n] += add
        rem -= add
        i += 1
    assert sum(tiles) == cap and all(256 <= t <= 384 for t in tiles), (cap, tiles)
    # ascending: the last (largest) tile maximizes the compute window that
    # hides the next slot's weight prefetch
    return sorted(tiles)


def build_program(caps, mode=MM_MODE, loop_reps=1):
    """Build the per-core program. caps: per-slot token capacities
    (len EPC). loop_reps>1 wraps the body in a hardware For_i loop
    (identical work each iteration) for wall-clock timing."""
    import contextlib

    import concourse.mybir as mybir
    import concourse.tile as tile
    from concourse import bacc

    f32 = mybir.dt.float32
    if mode == "fp32":
        act_dt = w_dt = f32
    elif mode == "fp32r":
        act_dt = w_dt = mybir.dt.float32r
    elif mode == "bf16":
        act_dt = w_dt = mybir.dt.bfloat16
    else:
        raise ValueError(mode)

    nc = bacc.Bacc("TRN2", target_bir_lowering=False, debug=False)
    xts, w1s, b1s, w2s, b2s, yts = [], [], [], [], [], []
    for s, cap in enumerate(caps):
        xts.append(
            nc.dram_tensor(f"xt{s}", [D_MODEL, cap], act_dt, kind="ExternalInput").ap()
        )
        w1s.append(
            nc.dram_tensor(
                f"w1{s}", [D_MODEL, D_HIDDEN], w_dt, kind="ExternalInput"
            ).ap()
        )
        b1s.append(
            nc.dram_tensor(f"b1{s}", [D_HIDDEN], f32, kind="ExternalInput").ap()
        )
        w2s.append(
            nc.dram_tensor(
                f"w2{s}", [D_HIDDEN, D_MODEL], w_dt, kind="ExternalInput"
            ).ap()
        )
        if L2B:
            # b2 replicated across partitions (bias varies along the free
            # dim in layout B, so it is added on the DVE, not the act bias)
            b2s.append(
                nc.dram_tensor(
                    f"b2r{s}", [128, D_MODEL], f32, kind="ExternalInput"
                ).ap()
            )
            tc_n = -(-cap // 128)
            yts.append(
                nc.dram_tensor(
                    f"yt{s}", [tc_n * 128, D_MODEL], f32, kind="ExternalOutput"
                ).ap()
            )
        else:
            b2s.append(
                nc.dram_tensor(f"b2{s}", [D_MODEL], f32, kind="ExternalInput").ap()
            )
            yts.append(
                nc.dram_tensor(
                    f"yt{s}", [D_MODEL, cap], f32, kind="ExternalOutput"
                ).ap()
            )

    Relu = mybir.ActivationFunctionType.Relu
    Ident = mybir.ActivationFunctionType.Identity

    HALF = MC2 // 2  # m2 chunks per W2 half-pool
    with tile.TileContext(nc) as tc:
        with (
            tc.tile_pool(name="w1pa", bufs=1) as w1pa,
            tc.tile_pool(name="w1pb", bufs=1) as w1pb,
            tc.tile_pool(name="w2pa", bufs=1) as w2pa,
            tc.tile_pool(name="w2pb", bufs=1) as w2pb,
            tc.tile_pool(name="bp", bufs=2) as bp,
            tc.tile_pool(name="bpr", bufs=2) as bpr,
            tc.tile_pool(name="xp", bufs=2) as xp,
            tc.tile_pool(name="hp", bufs=2) as hp,
            tc.tile_pool(name="yp", bufs=4) as yp,
            tc.tile_pool(name="ps1", bufs=2, space="PSUM") as ps1,
            tc.tile_pool(name="ps2", bufs=4 if L2B else 2, space="PSUM") as ps2,
        ):
            loop_cm = (
                tc.For_i(0, loop_reps, 1)
                if loop_reps > 1
                else contextlib.nullcontext()
            )
            with loop_cm:
                for e, cap in enumerate(caps):
                    tiles = _split_tiles(cap)
                    xt_src = xts[e].rearrange("(c p) n -> p c n", p=128)
                    if L2B:
                        yt_dst = yts[e].rearrange("(t p) d -> p t d", p=128)
                    else:
                        yt_dst = yts[e].rearrange("(c p) n -> p c n", p=128)

                    # The DMA fabric drains transfers roughly in enqueue
                    # order, so emissions follow need order. W1: two
                    # half-pools (m 0..7 / 8..15) x quarter-DMAs each on
                    # sync; quarters keep startup short, half-pools release
                    # early so the next slot's W1 hides under compute.
                    # Biases slot between quarters (tiny, needed early).
                    w1_src = w1s[e].rearrange("(c p) m -> p c m", p=128)
                    HW1 = D_HIDDEN // 2
                    w1ta = w1pa.tile([128, KC1, HW1], w_dt, tag="w1ta")
                    w1tb = w1pb.tile([128, KC1, HW1], w_dt, tag="w1tb")
                    NQ1 = 4  # DMAs per W1 half-pool
                    EW = HW1 // NQ1
                    nc.sync.dma_start(w1ta[:, :, :EW], w1_src[:, :, :EW])
                    b1t = bp.tile([128, MC1], f32, tag="b1t")
                    nc.sync.dma_start(b1t[:], b1s[e].rearrange("(m p) -> p m", p=128))
                    if L2B:
                        b2t = bpr.tile([128, D_MODEL], f32, tag="b2rt")
                        nc.sync.dma_start(b2t[:], b2s[e])
                    else:
                        b2t = bp.tile([128, MC2], f32, tag="b2t")
                        nc.sync.dma_start(
                            b2t[:], b2s[e].rearrange("(m p) -> p m", p=128)
                        )
                    for q in range(1, NQ1):
                        nc.sync.dma_start(
                            w1ta[:, :, q * EW : (q + 1) * EW],
                            w1_src[:, :, q * EW : (q + 1) * EW],
                        )
                    for q in range(NQ1):
                        nc.sync.dma_start(
                            w1tb[:, :, q * EW : (q + 1) * EW],
                            w1_src[:, :, HW1 + q * EW : HW1 + (q + 1) * EW],
                        )

                    # gpsimd queue in need order: xt[0], W2 quarters (into
                    # two half-pools, so the next slot's halves load under
                    # this slot's L2), then xt[1..].
                    w2_src = w2s[e].rearrange("(c p) m -> p c m", p=128)
                    off = [sum(tiles[:j]) for j in range(len(tiles))]
                    xtiles = []
                    for j, nt in enumerate(tiles):
                        xtile_j = xp.tile(
                            [128, KC1, nt], act_dt, tag="xtile", name=f"xtile_{e}_{j}"
                        )
                        xtiles.append(xtile_j)
                    # xt[0] split by k-chunk pairs: the first L1 matmul only
                    # needs k-chunk 0, so it starts early.
                    for cc in range(0, KC1, 2):
                        nc.gpsimd.dma_start(
                            xtiles[0][:, cc : cc + 2, :],
                            xt_src[:, cc : cc + 2, off[0] : off[0] + tiles[0]],
                        )
                    if len(tiles) > 1:
                        nc.gpsimd.dma_start(
                            xtiles[1][:], xt_src[:, :, off[1] : off[1] + tiles[1]]
                        )
                    HW2 = HALF * 128
                    w2ta = w2pa.tile([128, KC2, HW2], w_dt, tag="w2ta")
                    w2tb = w2pb.tile([128, KC2, HW2], w_dt, tag="w2tb")
                    NQ2 = 4  # DMAs per W2 half-pool
                    QW2 = HW2 // NQ2
                    for q in range(NQ2):
                        nc.gpsimd.dma_start(
                            w2ta[:, :, q * QW2 : (q + 1) * QW2],
                            w2_src[:, :, q * QW2 : (q + 1) * QW2],
                        )
                    for q in range(NQ2):
                        nc.gpsimd.dma_start(
                            w2tb[:, :, q * QW2 : (q + 1) * QW2],
                            w2_src[:, :, HW2 + q * QW2 : HW2 + (q + 1) * QW2],
                        )
                    for j in range(2, len(tiles)):
                        nc.gpsimd.dma_start(
                            xtiles[j][:], xt_src[:, :, off[j] : off[j] + tiles[j]]
                        )

                    # j-level software pipeline: L1(0), L1(1), L2(0...),
                    # L1(2), L2(...) — PE is in-order, so emitting the
                    # next tile's L1 before this tile's L2 lets layer-1 run
                    # while W2 is still streaming in.
                    T = len(tiles)
                    if L2B:
                        # unified ht per slot: L2B's 128-token lhsT slices
                        # cross j-tile boundaries
                        ht_u = hp.tile(
                            [128, KC2, cap], act_dt, tag="ht", name=f"ht_{e}"
                        )
                        hts = [None] * T
                    else:
                        hts = [None] * T

                    def layer1(j):
                        nt = tiles[j]
                        if L2B:
                            ht = None
                        else:
                            ht = hp.tile(
                                [128, KC2, nt], act_dt, tag="ht", name=f"ht_{e}_{j}"
                            )
                            hts[j] = ht
                        for m in range(MC1):
                            w1h = w1ta if m < MC1 // 2 else w1tb
                            mh = m % (MC1 // 2)
                            hps = ps1.tile([128, nt], f32, tag="hps")
                            for c in range(KC1):
                                nc.tensor.matmul(
                                    hps[:],
                                    lhsT=w1h[:, c, mh * 128 : (mh + 1) * 128],
                                    rhs=xtiles[j][:, c, :],
                                    start=(c == 0),
                                    stop=(c == KC1 - 1),
                                )
                            if L2B:
                                nc.scalar.activation(
                                    ht_u[:, m, off[j] : off[j] + nt],
                                    hps[:],
                                    Relu,
                                    bias=b1t[:, m : m + 1],
                                )
                            else:
                                nc.scalar.activation(
                                    ht[:, m, :], hps[:], Relu, bias=b1t[:, m : m + 1]
                                )

                    def layer2(j):
                        nt = tiles[j]
                        ht = hts[j]
                        for m in range(MC2):
                            w2h = w2ta if m < HALF else w2tb
                            mh = m % HALF
                            yps = ps2.tile([128, nt], f32, tag="yps")
                            for c in range(KC2):
                                nc.tensor.matmul(
                                    yps[:],
                                    lhsT=w2h[:, c, mh * 128 : (mh + 1) * 128],
                                    rhs=ht[:, c, :],
                                    start=(c == 0),
                                    stop=(c == KC2 - 1),
                                )
                            ysb = yp.tile([128, nt], f32, tag="ysb")
                            nc.scalar.activation(
                                ysb[:], yps[:], Ident, bias=b2t[:, m : m + 1]
                            )
                            nc.scalar.dma_start(
                                yt_dst[:, m, off[j] : off[j] + nt], ysb[:]
                            )

                    def layer2b(tc_list):
                        # layout B: out[tokens, d] — lhsT = ht 128-token
                        # slice, rhs = W2 rows, moving dim = d (512-wide)
                        Add = mybir.AluOpType.add
                        Mult = mybir.AluOpType.mult
                        for tcn in tc_list:
                            pw = min(128, cap - tcn * 128)
                            for dseg in range(2):
                                w2h = w2ta if dseg == 0 else w2tb
                                yps = ps2.tile([128, 512], f32, tag="yps")
                                for hk in range(KC2):
                                    nc.tensor.matmul(
                                        yps[:pw, :],
                                        lhsT=ht_u[:, hk, tcn * 128 : tcn * 128 + pw],
                                        rhs=w2h[:, hk, :],
                                        start=(hk == 0),
                                        stop=(hk == KC2 - 1),
                                    )
                                ysb = yp.tile([128, 512], f32, tag="ysb")
                                nc.vector.scalar_tensor_tensor(
                                    ysb[:pw, :],
                                    yps[:pw, :],
                                    1.0,
                                    b2t[:pw, dseg * 512 : (dseg + 1) * 512],
                                    Mult,
                                    Add,
                                )
                                nc.scalar.dma_start(
                                    yt_dst[:pw, tcn, dseg * 512 : (dseg + 1) * 512],
                                    ysb[:pw, :],
                                )

                    if L2B:
                        TC = -(-cap // 128)
                        # tc chunk usable once L1 has covered its tokens
                        cum = 0
                        ready_after = []  # per j: last tc (exclusive) ready
                        for j, nt in enumerate(tiles):
                            cum += nt
                            ready_after.append(
                                TC if cum == cap else cum // 128
                            )
                        emitted = 0
                        for j in range(T):
                            layer1(j)
                            if j >= 1:
                                layer2b(range(emitted, ready_after[j - 1]))
                                emitted = ready_after[j - 1]
                        layer2b(range(emitted, TC))
                    else:
                        for k in range(T + 1):
                            if k < T:
                                layer1(k)
                            if k >= 1:
                                layer2(k - 1)
    nc.compile()
    return nc


def route(x, Wg):
    """Host router identical (up to fp rounding far below the top-2/3
    logit gap) to the reference: top-2 by logit, softmax over the pair."""
    logits = x.astype(np.float32, copy=False) @ Wg.astype(np.float32, copy=False).T
    n = logits.shape[0]
    rows = np.arange(n)
    i1 = np.argmax(logits, axis=1)
    v1 = logits[rows, i1]
    masked = logits.copy()
    masked[rows, i1] = -np.inf
    i2 = np.argmax(masked, axis=1)
    v2 = masked[rows, i2]
    d = np.exp((v2 - v1).astype(np.float64))
    wt1 = (1.0 / (1.0 + d)).astype(np.float32)
    wt2 = (d / (1.0 + d)).astype(np.float32)
    return i1, i2, wt1, wt2


def plan(x, Wg):
    """Route, then assign experts to (core, slot) balanced by load.

    Returns (idxs, wts, slot_expert, caps):
      idxs[e], wts[e]: token ids / combine weights per expert
      slot_expert[core][s]: expert id in slot s of core
      caps[s]: token capacity of slot s (same across cores)
    """
    i1, i2, wt1, wt2 = route(x, Wg)
    idxs, wts = [], []
    for e in range(N_EXPERTS):
        sel1 = i1 == e
        sel2 = i2 == e
        idx = np.concatenate([np.nonzero(sel1)[0], np.nonzero(sel2)[0]])
        w = np.concatenate([wt1[sel1], wt2[sel2]])
        idxs.append(idx)
        wts.append(w)

    loads = np.array([len(i) for i in idxs])
    order = np.argsort(-loads)  # experts by load, desc
    slot_expert = [[0] * EPC for _ in range(N_CORES)]
    caps = []
    for s in range(EPC):
        grp = order[s * N_CORES : (s + 1) * N_CORES]
        for core in range(N_CORES):
            slot_expert[core][s] = int(grp[core])
        cap = int(loads[grp].max())
        cap = max(256, -(-cap // 16) * 16)  # >=256 (tile floor), mult of 16
        caps.append(cap)
    return idxs, wts, slot_expert, caps


def make_in_maps(x, W1, b1, W2, b2, idxs, slot_expert, caps, mode=None):
    import ml_dtypes

    if mode is None:
        mode = MM_MODE
    np_dt = ml_dtypes.bfloat16 if mode == "bf16" else np.float32
    in_maps = []
    for core in range(N_CORES):
        m = {}
        for s, cap in enumerate(caps):
            e = slot_expert[core][s]
            xt = np.zeros((D_MODEL, cap), dtype=np_dt)
            xt[:, : len(idxs[e])] = x[idxs[e]].T.astype(np_dt)
            m[f"xt{s}"] = xt
            m[f"w1{s}"] = np.ascontiguousarray(W1[e]).astype(np_dt)
            m[f"b1{s}"] = np.ascontiguousarray(b1[e])
            m[f"w2{s}"] = np.ascontiguousarray(W2[e]).astype(np_dt)
            if L2B:
                m[f"b2r{s}"] = np.ascontiguousarray(
                    np.broadcast_to(b2[e], (128, D_MODEL)).astype(np.float32)
                )
            else:
                m[f"b2{s}"] = np.ascontiguousarray(b2[e])
        in_maps.append(m)
    return in_maps


def kernel(x, Wg, W1, b1, W2, b2):
    from concourse.bass_utils import run_bass_kernel_spmd

    x = np.ascontiguousarray(np.asarray(x, dtype=np.float32))
    Wg = np.asarray(Wg, dtype=np.float32)
    W1 = np.asarray(W1, dtype=np.float32)
    b1 = np.asarray(b1, dtype=np.float32)
    W2 = np.asarray(W2, dtype=np.float32)
    b2 = np.asarray(b2, dtype=np.float32)
    n_tokens = x.shape[0]

    idxs, wts, slot_expert, caps = plan(x, Wg)
    in_maps = make_in_maps(x, W1, b1, W2, b2, idxs, slot_expert, caps)

    nc = build_program(caps)
    res = run_bass_kernel_spmd(nc, in_maps, core_ids=list(range(N_CORES)))

    out = np.zeros((n_tokens, D_MODEL), dtype=np.float32)
    for core in range(N_CORES):
        for s in range(EPC):
            e = slot_expert[core][s]
            n_e = len(idxs[e])
            if n_e == 0:
                continue
            if L2B:
                y = res.results[core][f"yt{s}"][:n_e]  # [n_e, D]
            else:
                y = res.results[core][f"yt{s}"][:, :n_e].T  # [n_e, D]
            out[idxs[e]] += wts[e][:, None] * y
    return out


if __name__ == "__main__":
    rng = np.random.default_rng(0)
    x = rng.standard_normal((N_TOKENS, D_MODEL), dtype=np.float32)
    s_in = 1.0 / np.sqrt(D_MODEL)
    s_hid = 1.0 / np.sqrt(D_HIDDEN)
    Wg = rng.uniform(-s_in, s_in, (N_EXPERTS, D_MODEL)).astype(np.float32)
    W1 = rng.uniform(-s_in, s_in, (N_EXPERTS, D_MODEL, D_HIDDEN)).astype(np.float32)
    b1 = rng.uniform(-s_in, s_in, (N_EXPERTS, D_HIDDEN)).astype(np.float32)
    W2 = rng.uniform(-s_hid, s_hid, (N_EXPERTS, D_HIDDEN, D_MODEL)).astype(np.float32)
    b2 = rng.uniform(-s_hid, s_hid, (N_EXPERTS, D_MODEL)).astype(np.float32)
    t0 = time.time()
    out = kernel(x=x, Wg=Wg, W1=W1, b1=b1, W2=W2, b2=b2)
    print("kernel() wall:", time.time() - t0, "out", out.shape, out.dtype)
